# revision 1
# baseline (speedup 1.0000x reference)
"""Sliding-window multi-head attention (N=4, T=2048, D=1024, H=16, hd=64,
rotary over all 64 dims, window (128,128)) on 8 Trainium2 NeuronCores.

Sharding: data-parallel over (batch, sequence-half): core c handles batch
c//2, query tokens [h*1024, (h+1)*1024) with a 128-token KV halo on each
side (zero-padded at sequence edges, masked in softmax).

Per-core device program (SPMD, one NEFF):
  P1  qkv projection: qT,kT feature-major [feat, tok], V token-major.
  P2  RoPE fused into P1: swap halves via a PE permutation matmul, combine
      with host-precomputed cos / signed-sin tables on DVE.
  P3  banded attention: scoresT tiles [128 kt, 512 qt] (one fp32r matmul),
      ACT exp(scale=1/8) -> bf16 probs, DVE band-mask multiply, bf16 AV
      matmul accumulating [64, 512] + ones-vector matmul for softmax sums.
      Normalize: DVE reciprocal + PE ones-broadcast + DVE multiply.
  P4  output projection (fp32r), DMA yT [1024 feat, 1024 tok] out.

Host: shard/pad/transpose inputs, build rope/mask tables, reassemble the
[4, 2048, 1024] output (+bout).
"""

import math

import ml_dtypes
import numpy as np

import bass_rust
import concourse.bass as bass
import concourse.mybir as mybir
import concourse.tile as tile
from concourse.bass_utils import run_bass_kernel_spmd
from concourse.vector_clock import ScopedClock

# ----------------------------------------------------------------------------
# Problem constants (hardcoded per the harness contract)
N, T, D = 4, 2048, 1024
H, HD = 16, 64
WINDOW = 128
ROPE_BASE = 10000.0
SCALE = 1.0 / math.sqrt(HD)

NCORES = 8
TQ = 1024             # query tokens per core
TE = TQ + 2 * WINDOW  # 1280 extended kv tokens per core
QB = 512              # query block
NQB = TQ // QB        # 2
KB = 128              # key block
NKB = (QB + 2 * WINDOW) // KB  # 6 key blocks per query block

VS = HD + 1  # per-head column stride in V (col 64 = ones)
VW = 16 * VS + 64  # padded so lhsT [*, 65h:65h+128] stays in range

F32 = mybir.dt.float32
F32R = mybir.dt.float32r
BF16 = mybir.dt.bfloat16

_MAXW = 1  # this container's walrus accepts one sync wait per instruction


class SplitWaitTC(tile.TileContext):
    """TileContext that spreads multi-sem waits over NoOp carriers so every
    instruction carries at most one sync wait (codegen limit here)."""

    _waitnop_counter = 0

    def _split_waits(self, inst, commit):
        si = getattr(inst, "sync_info", None)
        if si is None:
            return
        waits = list(si.on_wait)
        if len(waits) <= _MAXW:
            return
        ups = list(si.on_update)
        head, keep = waits[:-_MAXW], waits[-_MAXW:]
        for w in head:
            nop = bass_rust.InstNoOp()
            nop.engine = inst.engine
            SplitWaitTC._waitnop_counter += 1
            nop.name = f"I-waitnop-{SplitWaitTC._waitnop_counter}"
            nop.bass_nofuse = True
            nop.sync_info = bass_rust.SyncInfo(on_wait=[w], on_update=[])
            commit(nop)
        inst.sync_info = bass_rust.SyncInfo(on_wait=keep, on_update=ups)

    def _commit_and_lower(self, inst, original_block, old_bb_map, bb_to_exit_bb):
        if isinstance(inst, mybir.Instruction) and not isinstance(
            inst, (tile.BassTileRelease,)
        ):
            self._split_waits(
                inst,
                lambda nop: super(SplitWaitTC, self)._commit_and_lower(
                    nop, original_block, old_bb_map, bb_to_exit_bb
                ),
            )
        return super()._commit_and_lower(inst, original_block, old_bb_map, bb_to_exit_bb)

    def _drain_and_barrier(self, tick_clock, wait_clock):
        probe = self.nc.sync.nop(nofuse=True)
        wait_clock.add_sem_waits(probe.ins, ScopedClock({None: tick_clock.global_clock}))
        si = probe.ins.sync_info
        waits = list(si.on_wait) if si is not None else []
        ups = list(si.on_update) if si is not None else []
        if len(waits) > _MAXW:
            probe.ins.sync_info = bass_rust.SyncInfo(on_wait=waits[:_MAXW], on_update=ups)
            rest = waits[_MAXW:]
            while rest:
                chunk, rest = rest[:_MAXW], rest[_MAXW:]
                n = self.nc.sync.nop(nofuse=True)
                n.ins.sync_info = bass_rust.SyncInfo(on_wait=chunk, on_update=[])
        self.nc.sync.drain()
        self.nc.all_engine_barrier()
        assert self.sems is not None
        popped = self.nc._tile_sem_poison_stack.pop()
        assert popped is self._sem_poison
        self.nc.clear_and_free_semaphores(list(self.sems.allocated().values()))
        self.nc.all_engine_barrier()


# ----------------------------------------------------------------------------
# Device program





def build_nc():
    nc = bass.Bass("TRN2", target_bir_lowering=False, debug=False, num_devices=NCORES)

    xt = nc.declare_dram_parameter("xt", [D, TE], F32R, isOutput=False)
    wqkv = nc.declare_dram_parameter("wqkv", [D, 3 * D], F32R, isOutput=False)
    wout = nc.declare_dram_parameter("wout", [D, D], F32R, isOutput=False)
    cq = nc.declare_dram_parameter("cq", [128, TQ], F32, isOutput=False)
    sq = nc.declare_dram_parameter("sq", [128, TQ], F32, isOutput=False)
    ck = nc.declare_dram_parameter("ck", [128, TE], F32, isOutput=False)
    sk = nc.declare_dram_parameter("sk", [128, TE], F32, isOutput=False)
    maskd = nc.declare_dram_parameter("mask", [128, NQB * NKB * QB], BF16, isOutput=False)
    permd = nc.declare_dram_parameter("perm", [128, 128], F32R, isOutput=False)
    onesd = nc.declare_dram_parameter("onesc", [128, HD], F32R, isOutput=False)
    onesbd = nc.declare_dram_parameter("onesb", [128, 1], BF16, isOutput=False)
    identd = nc.declare_dram_parameter("ident", [128, 128], F32R, isOutput=False)
    yt = nc.declare_dram_parameter("yt", [D, TQ], F32, isOutput=True)

    AF = mybir.ActivationFunctionType

    with nc.allow_low_precision(reason="fp32r feeds PE; fp32 accumulate"), SplitWaitTC(nc) as tc:
        with (
            tc.tile_pool(name="const", bufs=1) as constp,
            tc.tile_pool(name="persist", bufs=1) as persist,
        ):
            # constants
            perm_t = constp.tile([128, 128], F32R, name="perm", tag="perm")
            nc.sync.dma_start(perm_t[:], permd[:])
            ones_t = constp.tile([128, HD], F32R, name="ones", tag="ones")
            nc.sync.dma_start(ones_t[:], onesd[:])
            onesb_t = constp.tile([128, 1], BF16, name="onesb", tag="onesb")
            nc.sync.dma_start(onesb_t[:], onesbd[:])
            ident_t = constp.tile([128, 128], F32R, name="ident", tag="ident")
            nc.sync.dma_start(ident_t[:], identd[:])
            zbf_t = constp.tile([128, 128], BF16, name="zbf", tag="zbf")
            nc.vector.memset(zbf_t[:], 0.0)

            # persistent activations
            qT = [persist.tile([128, TQ], F32R, name=f"qT{i}", tag=f"qT{i}") for i in range(8)]
            kT = [persist.tile([128, TE], F32R, name=f"kT{i}", tag=f"kT{i}") for i in range(8)]
            vp = [persist.tile([128, VW], BF16, name=f"vp{i}", tag=f"vp{i}") for i in range(10)]
            aT = [persist.tile([128, TQ], F32R, name=f"aT{i}", tag=f"aT{i}") for i in range(8)]

            # ---------------- P1+P2: qkv projection + rope for q, k ----------
            with (
                tc.tile_pool(name="xtp", bufs=1) as xtp,
                tc.tile_pool(name="p1ps", bufs=4, space="PSUM") as p1ps,
            ):
              # resident xT tiles [128 dmodel, 1280 tok], P1 scope only
              xts = [xtp.tile([128, TE], F32R, name=f"xt{i}", tag=f"xt{i}") for i in range(8)]
              for kt in range(8):
                  nc.sync.dma_start(
                      xts[kt][:, :640], xt[kt * 128 : (kt + 1) * 128, :640]
                  )
              for kt in range(8):
                  nc.sync.dma_start(
                      xts[kt][:, 640:], xt[kt * 128 : (kt + 1) * 128, 640:]
                  )
              with (
                tc.tile_pool(name="tabp", bufs=1) as tabp,
                tc.tile_pool(name="wq", bufs=10) as wpool,
                tc.tile_pool(name="swps", bufs=3, space="PSUM") as swps,
                tc.tile_pool(name="stage", bufs=2) as stage,
                tc.tile_pool(name="ropetmp", bufs=2) as ropetmp,
              ):
                cq_t = tabp.tile([128, TQ], F32, name="cq", tag="cq")
                nc.sync.dma_start(cq_t[:], cq[:])
                sq_t = tabp.tile([128, TQ], F32, name="sq", tag="sq")
                nc.sync.dma_start(sq_t[:], sq[:])
                ck_t = tabp.tile([128, TE], F32, name="ck", tag="ck")
                nc.sync.dma_start(ck_t[:], ck[:])
                sk_t = tabp.tile([128, TE], F32, name="sk", tag="sk")
                nc.sync.dma_start(sk_t[:], sk[:])
                for m in range(16):  # 8 q feature tiles then 8 k feature tiles
                    is_q = m < 8
                    dest = qT[m] if is_q else kT[m - 8]
                    ctab = cq_t if is_q else ck_t
                    stab = sq_t if is_q else sk_t
                    ntok = TQ if is_q else TE
                    # q tokens sit at ext columns [WINDOW, WINDOW+TQ)
                    xoff = WINDOW if is_q else 0
                    wtiles = []
                    for kt in range(8):
                        w = wpool.tile([128, 128], F32R, name="w", tag="w")
                        nc.sync.dma_start(
                            w[:], wqkv[kt * 128 : (kt + 1) * 128, m * 128 : (m + 1) * 128]
                        )
                        wtiles.append(w)
                    tb0 = 0
                    while tb0 < ntok:
                        nt = min(512, ntok - tb0)
                        ps = p1ps.tile([128, 512], F32, name="p1", tag="p1")
                        for kt in range(8):
                            nc.tensor.matmul(
                                ps[:, :nt],
                                (wtiles[kt][:]),
                                (xts[kt][:, xoff + tb0 : xoff + tb0 + nt]),
                                start=(kt == 0),
                                stop=(kt == 7),
                            )
                        raw = stage.tile([128, 512], F32R, name="raw", tag="raw")
                        nc.scalar.copy(raw[:, :nt], ps[:, :nt])
                        psw = swps.tile([128, 512], F32, name="sw", tag="sw")
                        nc.tensor.matmul(
                            psw[:, :nt], (perm_t[:]), (raw[:, :nt]),
                            start=True, stop=True,
                        )
                        t1 = ropetmp.tile([128, 512], F32, name="t1", tag="t1")
                        nc.gpsimd.tensor_mul(
                            t1[:, :nt], raw[:, :nt], ctab[:, tb0 : tb0 + nt]
                        )
                        t2 = ropetmp.tile([128, 512], F32, name="t2", tag="t2")
                        nc.vector.tensor_mul(
                            t2[:, :nt], psw[:, :nt], stab[:, tb0 : tb0 + nt]
                        )
                        nc.vector.tensor_add(
                            dest[:, tb0 : tb0 + nt], t1[:, :nt], t2[:, :nt]
                        )
                        tb0 += nt

              # V: token-major [tok, feat] bf16 — tt-outer so each V tile is
              # fully ready (both halves + ones) early for attention overlap
              with tc.tile_pool(name="wvp", bufs=17) as wvpool:
                wvtiles = {}
                for fb in range(2):
                    for kt in range(8):
                        w = wvpool.tile([128, 512], F32R, name="wv", tag="wv")
                        nc.sync.dma_start(
                            w[:],
                            wqkv[
                                kt * 128 : (kt + 1) * 128,
                                2 * D + fb * 512 : 2 * D + (fb + 1) * 512,
                            ],
                        )
                        wvtiles[(fb, kt)] = w
                for tt in range(10):
                    nc.gpsimd.memset(vp[tt][:], 0.0)
                    for fb in range(2):
                        ps = p1ps.tile([128, 512], F32, name="p1", tag="p1")
                        for kt in range(8):
                            nc.tensor.matmul(
                                ps[:],
                                (xts[kt][:, tt * 128 : (tt + 1) * 128]),
                                (wvtiles[(fb, kt)][:]),
                                start=(kt == 0),
                                stop=(kt == 7),
                            )
                        dst = vp[tt][:, : 16 * VS].rearrange("p (h s) -> p h s", s=VS)[
                            :, fb * 8 : (fb + 1) * 8, :HD
                        ]
                        nc.vector.tensor_copy(
                            dst, ps[:].rearrange("p (h s) -> p h s", s=HD)
                        )
                    onescols = vp[tt][:, : 16 * VS].rearrange(
                        "p (h s) -> p h s", s=VS
                    )[:, :, HD:]
                    nc.vector.memset(onescols, 1.0)

            # ---------------- P3: banded attention ---------------------------
            with (
                tc.tile_pool(name="maskp", bufs=1) as maskp,
                tc.tile_pool(name="sps", bufs=3, space="PSUM") as sps,
                tc.tile_pool(name="avps", bufs=3, space="PSUM") as avps,
                tc.tile_pool(name="bps", bufs=2, space="PSUM") as bps,
                tc.tile_pool(name="probs", bufs=8) as probsp,
                tc.tile_pool(name="smalls", bufs=4) as smalls,
            ):
                mask_t = maskp.tile([128, NQB * NKB * QB], BF16, name="mask", tag="mask")
                nc.sync.dma_start(mask_t[:], maskd[:])
                for qb in range(NQB):
                    for h in range(H):
                        ft = h // 2
                        p0 = (h % 2) * 64
                        psA = avps.tile([128, QB], F32, name="av", tag="av")
                        # full-width zero mm clears has_written so striped av
                        # mms can overwrite-then-accumulate per column
                        nc.tensor.matmul(
                            psA[:], zbf_t[:], vp[0][:, :QB],
                            start=True, stop=False, skip_group_check=True,
                        )
                        for kb in range(NKB):
                            kv0 = qb * QB + kb * KB  # ext row of first key
                            mc0 = (qb * NKB + kb) * QB
                            # valid query stripe for this key block
                            off = max(0, kb * KB - 2 * WINDOW)
                            end = min(QB, kb * KB + WINDOW)
                            w = end - off
                            psS = sps.tile([128, QB], F32, name="s", tag="s")
                            nc.tensor.matmul(
                                psS[:, :w],
                                (kT[ft][p0 : p0 + 64, kv0 : kv0 + KB]),
                                (qT[ft][p0 : p0 + 64, qb * QB + off : qb * QB + end]),
                                start=True,
                                stop=True,
                            )
                            pr = probsp.tile([128, QB], BF16, name="pr", tag="pr")
                            nc.scalar.activation(pr[:, :w], psS[:, :w], AF.Exp, scale=SCALE)
                            nc.vector.tensor_mul(
                                pr[:, :w], pr[:, :w], mask_t[:, mc0 + off : mc0 + end]
                            )
                            vt = kv0 // 128
                            nc.tensor.matmul(
                                psA[:, off:end],
                                vp[vt][:, h * VS : h * VS + 128],
                                pr[:, :w],
                                start=False,
                                stop=(kb == NKB - 1),
                                skip_group_check=True,
                            )
                        rc = smalls.tile([128, QB], F32R, name="rc", tag="rc")
                        nc.vector.reciprocal(rc[HD : HD + 1, :], psA[HD : HD + 1, :])
                        psB = bps.tile([128, QB], F32, name="b", tag="b")
                        nc.tensor.matmul(
                            psB[:HD, :],
                            (ones_t[HD : HD + 1, :]),
                            (rc[HD : HD + 1, :]),
                            start=True,
                            stop=True,
                        )
                        bc = smalls.tile([128, QB], F32, name="bc", tag="bc")
                        nc.scalar.copy(bc[:HD, :], psB[:HD, :])
                        if p0 == 0:
                            nc.vector.tensor_mul(
                                aT[ft][:HD, qb * QB : (qb + 1) * QB],
                                psA[:HD, :],
                                bc[:HD, :],
                            )
                        else:
                            an = smalls.tile([HD, QB], F32R, name="an", tag="an")
                            nc.vector.tensor_mul(an[:], psA[:HD, :], bc[:HD, :])
                            nc.sync.dma_start(
                                aT[ft][p0 : p0 + HD, qb * QB : (qb + 1) * QB], an[:]
                            )

            # ---------------- P4: output projection --------------------------
            with (
                tc.tile_pool(name="wo", bufs=18) as wop,
                tc.tile_pool(name="yps", bufs=2, space="PSUM") as yps,
                tc.tile_pool(name="yst", bufs=3) as yst,
            ):
                for mo in range(8):
                    wtiles = []
                    for kf in range(8):
                        w = wop.tile([128, 128], F32R, name="wo", tag="wo")
                        nc.sync.dma_start(
                            w[:], wout[kf * 128 : (kf + 1) * 128, mo * 128 : (mo + 1) * 128]
                        )
                        wtiles.append(w)
                    for q2 in range(NQB):
                        ps = yps.tile([128, QB], F32, name="y", tag="y")
                        for kf in range(8):
                            nc.tensor.matmul(
                                ps[:],
                                (wtiles[kf][:]),
                                (aT[kf][:, q2 * QB : (q2 + 1) * QB]),
                                start=(kf == 0),
                                stop=(kf == 7),
                            )
                        ys = yst.tile([128, QB], F32, name="ys", tag="ys")
                        nc.scalar.copy(ys[:], ps[:])
                        nc.sync.dma_start(
                            yt[mo * 128 : (mo + 1) * 128, q2 * QB : (q2 + 1) * QB], ys[:]
                        )

    return nc


# ----------------------------------------------------------------------------
# Host-side shard preparation


def _rope_tables(pos):
    """[128, len(pos)] cos and signed-sin tables for the 2-head tile layout."""
    inv_freq = 1.0 / (ROPE_BASE ** (np.arange(0, HD, 2, dtype=np.float32) / HD))  # [32]
    freqs = np.outer(pos.astype(np.float32), inv_freq)  # [T, 32]
    c32 = np.cos(freqs).astype(np.float32).T  # [32, T]
    s32 = np.sin(freqs).astype(np.float32).T
    ctab = np.tile(c32, (4, 1))  # rows r use freq r%32
    sgn = np.repeat(np.array([-1.0, 1.0, -1.0, 1.0], dtype=np.float32), 32)
    stab = np.tile(s32, (4, 1)) * sgn[:, None]
    return np.ascontiguousarray(ctab), np.ascontiguousarray(stab)


def _perm_matrix():
    p = np.zeros((128, 128), dtype=np.float32)
    for i in range(128):
        j = i + 32 if (i // 32) % 2 == 0 else i - 32
        p[i, j] = 1.0
    return p


def _core_inputs(x, Wqkv, Wout, core):
    n, half = core // 2, core % 2
    q0 = half * TQ            # first query token (global)
    e0 = q0 - WINDOW          # first ext kv token (global, may be negative)

    x_ext = np.zeros((TE, D), dtype=np.float32)
    lo, hi = max(e0, 0), min(e0 + TE, T)
    x_ext[lo - e0 : hi - e0] = x[n, lo:hi]
    xt = np.ascontiguousarray(x_ext.T)

    pos_q = np.arange(q0, q0 + TQ)
    pos_k = np.clip(np.arange(e0, e0 + TE), 0, T - 1)
    cqt, sqt = _rope_tables(pos_q)
    ckt, skt = _rope_tables(pos_k)

    # mask [128 kt, NQB*NKB*QB qt] in scoresT orientation
    mask = np.zeros((128, NQB * NKB * QB), dtype=np.float32)
    for qb in range(NQB):
        for kb in range(NKB):
            jj = e0 + qb * QB + kb * KB + np.arange(KB)  # global key index
            ii = q0 + qb * QB + np.arange(QB)            # global query index
            valid = (
                (np.abs(jj[:, None] - ii[None, :]) <= WINDOW)
                & (jj[:, None] >= 0)
                & (jj[:, None] < T)
            )
            mask[:, (qb * NKB + kb) * QB : (qb * NKB + kb + 1) * QB] = valid
    return {
        "xt": xt,
        "wqkv": np.ascontiguousarray(Wqkv, dtype=np.float32),
        "wout": np.ascontiguousarray(Wout, dtype=np.float32),
        "cq": cqt,
        "sq": sqt,
        "ck": ckt,
        "sk": skt,
        "mask": mask.astype(ml_dtypes.bfloat16),
        "perm": _perm_matrix(),
        "onesc": np.ones((128, HD), dtype=np.float32),
        "onesb": np.ones((128, 1), dtype=ml_dtypes.bfloat16),
        "ident": np.eye(128, dtype=np.float32),
    }


_NC_CACHE = {}


def _get_nc():
    if "nc" not in _NC_CACHE:
        _NC_CACHE["nc"] = build_nc()
    return _NC_CACHE["nc"]


def kernel(x, Wqkv, Wout, bout, _trace=False, _trace_kwargs=None):
    x = np.asarray(x, dtype=np.float32)
    in_maps = [_core_inputs(x, Wqkv, Wout, c) for c in range(NCORES)]
    nc = _get_nc()
    kw = {}
    if _trace:
        kw = {"trace": True, "trace_kwargs": _trace_kwargs or {}}
    res = run_bass_kernel_spmd(nc, in_maps, core_ids=list(range(NCORES)), **kw)
    out = np.empty((N, T, D), dtype=np.float32)
    for c in range(NCORES):
        n, half = c // 2, c % 2
        out[n, half * TQ : (half + 1) * TQ] = res.results[c]["yt"].T
    out += np.asarray(bout, dtype=np.float32)[None, None, :]
    kernel._last_results = res
    return out



# revision 3
# speedup vs baseline: 1.6542x; 1.6542x over previous
"""Sliding-window multi-head attention (N=4, T=2048, D=1024, H=16, hd=64,
full-dim rotary, window (128,128)) on 8 Trainium2 NeuronCores.

Sharding: (batch, head-half): core c handles batch c//2 and heads
[8*(c%2), 8*(c%2)+8) over the FULL sequence — no halo recompute. Each core
emits a partial out-projection (contraction over its 512 features); the host
sums the two partials per batch and adds bout.

Per-core program (all matmuls bf16, fp32 PSUM accumulate):
  P1  qkv projection from feature-major x; RoPE on q,k via a PE
      half-swap permutation matmul + cos/signed-sin tables.
      V token-major with per-head [64 v | 64 ones] column blocks.
  P3  banded attention per (head, 512-query block): score stripes
      [128 keys x <=384 queries] packed into one [128,1536] PSUM tile
      (bank-aligned), single exp, single band-mask multiply, ordered-start
      AV accumulation -> psA = [64 attnout rows | 64 replicated sum rows],
      one divide normalizes.
  P4  partial out-projection interleaved per query block; bf16 output.
"""

import math

import ml_dtypes
import numpy as np

import bass_rust
import concourse.bass as bass
import concourse.mybir as mybir
import concourse.tile as tile
from concourse.bass_utils import run_bass_kernel_spmd
from concourse.vector_clock import ScopedClock

# ----------------------------------------------------------------------------
N, T, D = 4, 2048, 1024
H, HD = 16, 64
HLOC = 8            # heads per core
FH = HLOC * HD      # 512 q/k/v features per core
WIN = 128
ROPE_BASE = 10000.0
SCALE = 1.0 / math.sqrt(HD)

NCORES = 8
QB = 512
NQB = T // QB       # 4
NKT = T // 128      # 16
SW = 1536           # packed score-tile width (3 PSUM banks)

F32 = mybir.dt.float32
BF16 = mybir.dt.bfloat16

# normalization implementation: "dve_mixed" | "pool_mixed" | "dma_align"
NORM_MODE = "dve_mixed"

_MAXW = 1  # this container's walrus accepts one sync wait per instruction


class SplitWaitTC(tile.TileContext):
    """TileContext that spreads multi-sem waits over NoOp carriers so every
    instruction carries at most one sync wait (codegen limit here)."""

    _waitnop_counter = 0

    def _split_waits(self, inst, commit):
        si = getattr(inst, "sync_info", None)
        if si is None:
            return
        waits = list(si.on_wait)
        if len(waits) <= _MAXW:
            return
        ups = list(si.on_update)
        head, keep = waits[:-_MAXW], waits[-_MAXW:]
        for w in head:
            nop = bass_rust.InstNoOp()
            nop.engine = inst.engine
            SplitWaitTC._waitnop_counter += 1
            nop.name = f"I-waitnop-{SplitWaitTC._waitnop_counter}"
            nop.bass_nofuse = True
            nop.sync_info = bass_rust.SyncInfo(on_wait=[w], on_update=[])
            commit(nop)
        inst.sync_info = bass_rust.SyncInfo(on_wait=keep, on_update=ups)

    def _commit_and_lower(self, inst, original_block, old_bb_map, bb_to_exit_bb):
        if isinstance(inst, mybir.Instruction) and not isinstance(
            inst, (tile.BassTileRelease,)
        ):
            self._split_waits(
                inst,
                lambda nop: super(SplitWaitTC, self)._commit_and_lower(
                    nop, original_block, old_bb_map, bb_to_exit_bb
                ),
            )
        return super()._commit_and_lower(inst, original_block, old_bb_map, bb_to_exit_bb)

    def _drain_and_barrier(self, tick_clock, wait_clock):
        probe = self.nc.sync.nop(nofuse=True)
        wait_clock.add_sem_waits(probe.ins, ScopedClock({None: tick_clock.global_clock}))
        si = probe.ins.sync_info
        waits = list(si.on_wait) if si is not None else []
        ups = list(si.on_update) if si is not None else []
        if len(waits) > _MAXW:
            probe.ins.sync_info = bass_rust.SyncInfo(on_wait=waits[:_MAXW], on_update=ups)
            rest = waits[_MAXW:]
            while rest:
                chunk, rest = rest[:_MAXW], rest[_MAXW:]
                n = self.nc.sync.nop(nofuse=True)
                n.ins.sync_info = bass_rust.SyncInfo(on_wait=chunk, on_update=[])
        self.nc.sync.drain()
        self.nc.all_engine_barrier()
        assert self.sems is not None
        popped = self.nc._tile_sem_poison_stack.pop()
        assert popped is self._sem_poison
        self.nc.clear_and_free_semaphores(list(self.sems.allocated().values()))
        self.nc.all_engine_barrier()


# ----------------------------------------------------------------------------
# Static stripe planning (shared by device codegen and host mask builder)


def qb_stripes(qb):
    """Score stripes for query block qb: list of (kt, off, end) with
    queries [qb*QB+off, qb*QB+end) valid for key tile kt."""
    q0 = qb * QB
    res = []
    for kt in range(max(0, q0 // 128 - 1), min(NKT, q0 // 128 + 5)):
        off = max(0, 128 * (kt - 1) - q0)
        end = min(QB, 128 * (kt + 2) - q0)
        if end > off:
            res.append((kt, off, end))
    return res


def pack_stripes(stripes):
    """Place stripes in a [128, SW] tile without crossing 512-col PSUM bank
    boundaries. Returns (placed=[(kt,off,end,col)], holes=[(c0,c1)])."""
    banks = [[] for _ in range(SW // 512)]
    fill = [0] * (SW // 512)
    for kt, off, end in sorted(stripes, key=lambda s: -(s[2] - s[1])):
        w = end - off
        for b in range(len(banks)):
            if fill[b] + w <= 512:
                banks[b].append((kt, off, end, b * 512 + fill[b]))
                fill[b] += w
                break
        else:
            raise AssertionError("stripe packing overflow")
    placed = [s for b in banks for s in b]
    holes = [
        (b * 512 + fill[b], (b + 1) * 512)
        for b in range(len(banks))
        if fill[b] < 512
    ]
    return placed, holes


def av_plan(placed):
    """Order AV matmuls so every psA column gets start=True exactly once
    before accumulation. Returns [(kt, joff, jend, pcol, start)]."""
    cov = np.zeros(QB, bool)
    order, todo = [], list(placed)
    while todo:
        best, key = None, (-1, -1)
        for s in todo:
            new = int((~cov[s[1]:s[2]]).sum())
            k = (1 if new == s[2] - s[1] and new > 0 else 0, new)
            if k > key:
                best, key = s, k
        if key[1] <= 0:
            break
        order.append(best)
        todo.remove(best)
        cov[best[1]:best[2]] = True
    order += todo
    assert cov.all()
    cov2 = np.zeros(QB, bool)
    mms = []
    for kt, off, end, col in order:
        j = off
        while j < end:
            st = not cov2[j]
            j2 = j
            while j2 < end and cov2[j2] != st:
                j2 += 1
            mms.append((kt, j, j2, col + (j - off), st))
            j = j2
        cov2[off:end] = True
    return mms


QB_PLANS = []
for _qb in range(NQB):
    _placed, _holes = pack_stripes(qb_stripes(_qb))
    QB_PLANS.append((_placed, _holes, av_plan(_placed)))
MASK_VARIANT = {0: 0, 1: 1, 2: 1, 3: 2}


# ----------------------------------------------------------------------------
# Device program


def build_nc():
    nc = bass.Bass("TRN2", target_bir_lowering=False, debug=False, num_devices=NCORES)

    xt = nc.declare_dram_parameter("xt", [D, T], BF16, isOutput=False)
    wq = nc.declare_dram_parameter("wq", [D, FH], BF16, isOutput=False)
    wk = nc.declare_dram_parameter("wk", [D, FH], BF16, isOutput=False)
    wv = nc.declare_dram_parameter("wv", [D, FH], BF16, isOutput=False)
    wo = nc.declare_dram_parameter("wo", [FH, D], BF16, isOutput=False)
    ctabd = nc.declare_dram_parameter("ctab", [128, T], BF16, isOutput=False)
    stabd = nc.declare_dram_parameter("stab", [128, T], BF16, isOutput=False)
    maskd = nc.declare_dram_parameter("mask", [128, 3 * SW], BF16, isOutput=False)
    permd = nc.declare_dram_parameter("perm", [128, 128], BF16, isOutput=False)
    yt = nc.declare_dram_parameter("yt", [D, T], BF16, isOutput=True)

    AF = mybir.ActivationFunctionType
    ALU = mybir.AluOpType

    with nc.allow_low_precision(reason="bf16 matmul inputs; fp32 accumulate"), \
            SplitWaitTC(nc) as tc:
        with (
            tc.tile_pool(name="const", bufs=1) as constp,
            tc.tile_pool(name="persist", bufs=1) as persist,
        ):
            perm_t = constp.tile([128, 128], BF16, name="perm", tag="perm")
            nc.sync.dma_start(perm_t[:], permd[:])
            zbf_t = constp.tile([128, 128], BF16, name="zbf", tag="zbf")
            nc.vector.memset(zbf_t[:], 0.0)
            ctab_t = constp.tile([128, T], BF16, name="ctab", tag="ctab")
            nc.sync.dma_start(ctab_t[:], ctabd[:])
            stab_t = constp.tile([128, T], BF16, name="stab", tag="stab")
            nc.sync.dma_start(stab_t[:], stabd[:])
            mask_t = constp.tile([128, 3 * SW], BF16, name="mask", tag="mask")
            nc.sync.dma_start(mask_t[:], maskd[:])
            wo_t = constp.tile([128, 4 * D], BF16, name="wo", tag="wo")
            nc.sync.dma_start(
                wo_t[:].rearrange("p (a f) -> p a f", f=D),
                wo[:].rearrange("(a p) f -> p a f", p=128),
            )

            qT = [persist.tile([128, T], BF16, name=f"qT{i}", tag=f"qT{i}") for i in range(4)]
            kT = [persist.tile([128, T], BF16, name=f"kT{i}", tag=f"kT{i}") for i in range(4)]
            vp = [persist.tile([128, 1024], BF16, name=f"vp{i}", tag=f"vp{i}") for i in range(NKT)]
            aT = [persist.tile([128, T], BF16, name=f"aT{i}", tag=f"aT{i}") for i in range(4)]

            # ---------------- P1: qkv projection + rope ----------------------
            with (
                tc.tile_pool(name="xtp", bufs=1) as xtp,
                tc.tile_pool(name="wp", bufs=1) as wp,
                tc.tile_pool(name="rawp", bufs=3) as rawp,
                tc.tile_pool(name="ropet", bufs=4) as ropet,
                tc.tile_pool(name="psq", bufs=2, space="PSUM") as psqp,
                tc.tile_pool(name="psw", bufs=2, space="PSUM") as pswp,
                tc.tile_pool(name="psv", bufs=2, space="PSUM") as psvp,
            ):
                xts = [xtp.tile([128, 8 * QB], BF16, name=f"xt{c}", tag=f"xt{c}") for c in range(4)]
                wq_t = wp.tile([128, 8 * FH], BF16, name="wq", tag="wq")
                wk_t = wp.tile([128, 8 * FH], BF16, name="wk", tag="wk")
                wv_t = wp.tile([128, 8 * FH], BF16, name="wv", tag="wv")

                nc.sync.dma_start(
                    xts[0][:].rearrange("p (a t) -> p a t", t=QB),
                    xt[:].rearrange("(a p) t -> p a t", p=128)[:, :, :QB],
                )
                nc.sync.dma_start(
                    wq_t[:].rearrange("p (a f) -> p a f", f=FH),
                    wq[:].rearrange("(a p) f -> p a f", p=128),
                )
                nc.sync.dma_start(
                    wk_t[:].rearrange("p (a f) -> p a f", f=FH),
                    wk[:].rearrange("(a p) f -> p a f", p=128),
                )
                for c in range(1, 4):
                    nc.sync.dma_start(
                        xts[c][:].rearrange("p (a t) -> p a t", t=QB),
                        xt[:].rearrange("(a p) t -> p a t", p=128)[:, :, c * QB:(c + 1) * QB],
                    )
                nc.sync.dma_start(
                    wv_t[:].rearrange("p (a f) -> p a f", f=FH),
                    wv[:].rearrange("(a p) f -> p a f", p=128),
                )

                # q/k feature tiles with rope; perm-matmuls staggered one
                # chunk behind the projection matmuls to keep PE fed.
                pending = []

                def emit_tail(args):
                    is_q, f, c, psq, raw = args
                    dest = qT[f] if is_q else kT[f]
                    c0 = c * QB
                    psw = pswp.tile([128, QB], F32, name="psw", tag="psw")
                    nc.tensor.matmul(psw[:], perm_t[:], raw[:], start=True, stop=True)
                    t1 = ropet.tile([128, QB], BF16, name="t1", tag="t1")
                    nc.gpsimd.tensor_mul(t1[:], raw[:], ctab_t[:, c0:c0 + QB])
                    t2 = ropet.tile([128, QB], BF16, name="t2", tag="t2")
                    nc.vector.tensor_mul(t2[:], psw[:], stab_t[:, c0:c0 + QB])
                    nc.vector.tensor_add(dest[:, c0:c0 + QB], t1[:], t2[:])

                for m in range(8):
                    is_q = m < 4
                    f = m % 4
                    w_t = wq_t if is_q else wk_t
                    for c in range(4):
                        psq = psqp.tile([128, QB], F32, name="psq", tag="psq")
                        for kt8 in range(8):
                            nc.tensor.matmul(
                                psq[:],
                                w_t[:, kt8 * FH + f * 128: kt8 * FH + (f + 1) * 128],
                                xts[c][:, kt8 * QB:(kt8 + 1) * QB],
                                start=(kt8 == 0),
                                stop=(kt8 == 7),
                            )
                        raw = rawp.tile([128, QB], BF16, name="raw", tag="raw")
                        nc.scalar.copy(raw[:], psq[:])
                        if pending:
                            emit_tail(pending.pop())
                        pending.append((is_q, f, c, psq, raw))
                while pending:
                    emit_tail(pending.pop())

                # V token-major: per 128-token tile, per-head [64 v | 64 ones]
                for kt in range(NKT):
                    qtr, toff = kt // 4, (kt % 4) * 128
                    ones_view = vp[kt][:].rearrange("p (h s) -> p h s", s=128)[:, :, HD:]
                    nc.vector.memset(ones_view, 1.0)
                    psv = psvp.tile([128, FH], F32, name="psv", tag="psv")
                    for kt8 in range(8):
                        nc.tensor.matmul(
                            psv[:],
                            xts[qtr][:, kt8 * QB + toff: kt8 * QB + toff + 128],
                            wv_t[:, kt8 * FH:(kt8 + 1) * FH],
                            start=(kt8 == 0),
                            stop=(kt8 == 7),
                        )
                    nc.scalar.copy(
                        vp[kt][:].rearrange("p (h s) -> p h s", s=128)[:, :, :HD],
                        psv[:].rearrange("p (h s) -> p h s", s=HD),
                    )

            # ---------------- P3 + P4 interleaved ----------------------------
            with (
                tc.tile_pool(name="probs", bufs=4) as probsp,
                tc.tile_pool(name="ysp", bufs=2) as ysp,
                tc.tile_pool(name="srecp", bufs=2) as srecp,
                tc.tile_pool(name="sps", bufs=2, space="PSUM") as sps,
                tc.tile_pool(name="smallps", bufs=2, space="PSUM") as smallps,
            ):
                def attn_unit_front(h, qb):
                    """QK stripes + exp + mask for (head, query block)."""
                    placed, holes, _ = QB_PLANS[qb]
                    v = MASK_VARIANT[qb]
                    f, p0 = h // 2, (h % 2) * HD
                    q0 = qb * QB
                    psS = sps.tile([128, SW], F32, name="psS", tag="psS")
                    for kt, off, end, col in placed:
                        nc.tensor.matmul(
                            psS[:, col:col + end - off],
                            kT[f][p0:p0 + HD, kt * 128:(kt + 1) * 128],
                            qT[f][p0:p0 + HD, q0 + off:q0 + end],
                            start=True,
                            stop=True,
                            skip_group_check=True,
                        )
                    for c0, c1 in holes:
                        nc.tensor.matmul(
                            psS[:, c0:c1],
                            zbf_t[:],
                            zbf_t[:, : c1 - c0],
                            start=True,
                            stop=True,
                            skip_group_check=True,
                        )
                    probs = probsp.tile([128, SW], BF16, name="probs", tag="probs")
                    nc.scalar.activation(probs[:], psS[:], AF.Exp, scale=SCALE)
                    nc.vector.tensor_mul(
                        probs[:], probs[:], mask_t[:, v * SW:(v + 1) * SW]
                    )
                    return probs

                def attn_unit_back(h, qb, probs):
                    """AV accumulation + normalize for (head, query block)."""
                    _, _, mms = QB_PLANS[qb]
                    f, p1 = h // 2, (h % 2) * HD
                    q0 = qb * QB
                    psA = smallps.tile([128, QB], F32, name="psA", tag="small")
                    for i, (kt, j, j2, cs, st) in enumerate(mms):
                        nc.tensor.matmul(
                            psA[:, j:j2],
                            vp[kt][:, h * 128:(h + 1) * 128],
                            probs[:, cs:cs + (j2 - j)],
                            start=st,
                            stop=(i == len(mms) - 1),
                            skip_group_check=True,
                        )
                    dst = aT[f][p1:p1 + HD, q0:q0 + QB]
                    if NORM_MODE == "dve_mixed":
                        nc.vector.tensor_tensor(
                            dst, psA[:HD, :], psA[HD:128, :], ALU.divide
                        )
                    elif NORM_MODE == "pool_mixed":
                        nc.gpsimd.tensor_tensor(
                            dst, psA[:HD, :], psA[HD:128, :], ALU.divide
                        )
                    else:  # dma_align
                        srec = srecp.tile([HD, QB], F32, name="srec", tag="srec")
                        nc.sync.dma_start(srec[:], psA[HD:128, :])
                        if p1 == 0:
                            nc.vector.tensor_tensor(
                                dst, psA[:HD, :], srec[:], ALU.divide
                            )
                        else:
                            an = srecp.tile([HD, QB], BF16, name="an", tag="an")
                            nc.vector.tensor_tensor(
                                an[:], psA[:HD, :], srec[:], ALU.divide
                            )
                            nc.sync.dma_start(dst, an[:])

                for qb in range(NQB):
                    prev = None
                    for h in range(HLOC):
                        probs = attn_unit_front(h, qb)
                        if prev is not None:
                            attn_unit_back(prev[0], qb, prev[1])
                        prev = (h, probs)
                    attn_unit_back(prev[0], qb, prev[1])

                    ys = ysp.tile([128, 8 * QB], BF16, name="ys", tag="ys")
                    for mo in range(8):
                        psy = smallps.tile([128, QB], F32, name="psy", tag="small")
                        for kf in range(4):
                            nc.tensor.matmul(
                                psy[:],
                                wo_t[:, kf * D + mo * 128: kf * D + (mo + 1) * 128],
                                aT[kf][:, qb * QB:(qb + 1) * QB],
                                start=(kf == 0),
                                stop=(kf == 3),
                            )
                        nc.gpsimd.tensor_copy(ys[:, mo * QB:(mo + 1) * QB], psy[:])
                    nc.sync.dma_start(
                        yt[:].rearrange("(a p) t -> p a t", p=128)[:, :, qb * QB:(qb + 1) * QB],
                        ys[:].rearrange("p (a t) -> p a t", t=QB),
                    )

    return nc


# ----------------------------------------------------------------------------
# Host-side shard preparation


def _rope_tables():
    """[128, T] cos and signed-sin tables for the 2-head tile row layout."""
    inv_freq = 1.0 / (ROPE_BASE ** (np.arange(0, HD, 2, dtype=np.float32) / HD))
    freqs = np.outer(np.arange(T, dtype=np.float32), inv_freq)  # [T, 32]
    c32 = np.cos(freqs).astype(np.float32).T
    s32 = np.sin(freqs).astype(np.float32).T
    ctab = np.tile(c32, (4, 1))
    sgn = np.repeat(np.array([-1.0, 1.0, -1.0, 1.0], dtype=np.float32), 32)
    stab = np.tile(s32, (4, 1)) * sgn[:, None]
    return ctab, stab


def _perm_matrix():
    p = np.zeros((128, 128), dtype=np.float32)
    for i in range(128):
        j = i + 32 if (i // 32) % 2 == 0 else i - 32
        p[i, j] = 1.0
    return p


def _build_masks():
    m = np.zeros((128, 3 * SW), dtype=np.float32)
    for vi, qb in enumerate([0, 1, 3]):
        placed, _, _ = QB_PLANS[qb]
        q0 = qb * QB
        for kt, off, end, col in placed:
            kk = kt * 128 + np.arange(128)[:, None]
            qq = q0 + np.arange(off, end)[None, :]
            m[:, vi * SW + col: vi * SW + col + end - off] = (
                np.abs(kk - qq) <= WIN
            )
    return m


_BF = ml_dtypes.bfloat16


def _core_inputs(x, Wqkv, Wout, core):
    n, hg = core // 2, core % 2
    f0 = hg * FH
    common = _CORE_COMMON
    return {
        "xt": np.ascontiguousarray(x[n].T).astype(_BF),
        "wq": np.ascontiguousarray(Wqkv[:, f0:f0 + FH]).astype(_BF),
        "wk": np.ascontiguousarray(Wqkv[:, D + f0:D + f0 + FH]).astype(_BF),
        "wv": np.ascontiguousarray(Wqkv[:, 2 * D + f0:2 * D + f0 + FH]).astype(_BF),
        "wo": np.ascontiguousarray(Wout[f0:f0 + FH, :]).astype(_BF),
        **common,
    }


_CORE_COMMON = None


def _common_inputs():
    global _CORE_COMMON
    if _CORE_COMMON is None:
        ctab, stab = _rope_tables()
        _CORE_COMMON = {
            "ctab": ctab.astype(_BF),
            "stab": stab.astype(_BF),
            "mask": _build_masks().astype(_BF),
            "perm": _perm_matrix().astype(_BF),
        }
    return _CORE_COMMON


_NC_CACHE = {}


def _get_nc():
    if "nc" not in _NC_CACHE:
        _NC_CACHE["nc"] = build_nc()
    return _NC_CACHE["nc"]


def kernel(x, Wqkv, Wout, bout, _trace=False, _trace_kwargs=None):
    x = np.asarray(x, dtype=np.float32)
    Wqkv = np.asarray(Wqkv, dtype=np.float32)
    Wout = np.asarray(Wout, dtype=np.float32)
    _common_inputs()
    in_maps = [_core_inputs(x, Wqkv, Wout, c) for c in range(NCORES)]
    nc = _get_nc()
    kw = {}
    if _trace:
        kw = {"trace": True, "trace_kwargs": _trace_kwargs or {}}
    res = run_bass_kernel_spmd(nc, in_maps, core_ids=list(range(NCORES)), **kw)
    out = np.zeros((N, T, D), dtype=np.float32)
    for c in range(NCORES):
        n = c // 2
        out[n] += np.asarray(res.results[c]["yt"], dtype=np.float32).T
    out += np.asarray(bout, dtype=np.float32)[None, None, :]
    kernel._last_results = res
    return out
